# revision 14
# baseline (speedup 1.0000x reference)
"""Trainium2 Bass kernel for CNN backbone + top-2 MoE head (B=4096).

Data-parallel over 8 NeuronCores (512 images each). Convs are computed as
PE matmuls with split-bf16 (hi/lo) operands for fp32-grade accuracy:
  conv1: host-built quarter im2col (K=108: 4 row-quarters x 27 taps),
         M=128 (4 quarters x 32 out-ch); 3 split passes.
  conv2: row bands (K=128: 4 pooled rows x 32 ch), M=128 (2 out-rows x
         64 out-ch, yloc-major); 3 dx passes x 3 split terms; row-pool
         via DMA partition move + aligned max.
  conv3: 9-tap accumulation (K=64), M=128; 2 passes per tap via hi/lo
         stacking in partitions.
BN is folded into conv weights/biases host-side. Maxpools run on DVE via
strided tensor_max; gate + experts run in exact fp32 on the PE.
"""
import os
import numpy as np
import ml_dtypes

import concourse.bass as bass
import concourse.mybir as mybir
import concourse.tile as tile
from concourse import bacc
from concourse.bass_utils import run_bass_kernel_spmd
from concourse.masks import make_identity

F32 = mybir.dt.float32
BF16 = mybir.dt.bfloat16

N_CORES = 8
B_FULL = 4096
BC = B_FULL // N_CORES      # 512 images per core
MEGA = 32                   # images per pipeline chunk
NMEGA = BC // MEGA
BN_EPS = 1e-5

bf16 = ml_dtypes.bfloat16

_cache = {}
last_result = None


# ---------------------------------------------------------------- host prep

def _fold_bn(w, b, g, beta, mean, var):
    inv = g / np.sqrt(var + BN_EPS)
    wf = w * inv[:, None, None, None]
    bf_ = (b - mean) * inv + beta
    return wf.astype(np.float32), bf_.astype(np.float32)


def _split(a):
    hi = a.astype(bf16)
    lo = (a - hi.astype(np.float32)).astype(bf16)
    return hi, lo


def _arr1(w):
    """conv1 lhsT [108, 128]: p=(q*27 + c*9 + dy*3 + dx), m=(q*32 + o)."""
    out = np.zeros((108, 128), np.float32)
    for q in range(4):
        for c in range(3):
            for dy in range(3):
                for dx in range(3):
                    out[q * 27 + c * 9 + dy * 3 + dx, q * 32:(q + 1) * 32] = \
                        w[:, c, dy, dx]
    return out


def _arr2(w, dxi):
    """conv2 lhsT [128, 128]: p=(rr*32 + c), m=(yloc*64 + o)."""
    out = np.zeros((128, 128), np.float32)
    for rr in range(4):
        for c in range(32):
            for yloc in range(2):
                dy = rr - yloc
                if 0 <= dy <= 2:
                    out[rr * 32 + c, yloc * 64:(yloc + 1) * 64] = w[:, c, dy, dxi]
    return out


def _arr3(w, dy, dx):
    """conv3 per-tap lhsT [64, 128]: p=c, m=o."""
    return np.ascontiguousarray(w[:, :, dy, dx].T)  # [c, o]


def _build_xq(x):
    """Host-side conv1 quarter im2col: two [108, B, 256] bf16 (hi, lo).

    Partition p = q*27 + c*9 + dy*3 + dx; column n = ry*32 + xx within
    quarter q (out pixel y = q*8+ry); value = x[b, c, y+dy-1, xx+dx-1],
    zero-padded.
    """
    n = x.shape[0]
    xh = x.astype(bf16).astype(np.float32)
    xl = (x - xh).astype(bf16).astype(np.float32)
    outs = []
    for xv in (xh, xl):
        xpad = np.zeros((n, 3, 34, 34), np.float32)
        xpad[:, :, 1:33, 1:33] = xv
        xq = np.zeros((108, n, 256), np.float32)
        for q in range(4):
            for c in range(3):
                for dy in range(3):
                    for dx in range(3):
                        p = q * 27 + c * 9 + dy * 3 + dx
                        sl = xpad[:, c, q * 8 + dy:q * 8 + dy + 8, dx:dx + 32]
                        xq[p] = sl.reshape(n, 256)
        outs.append(xq.astype(bf16))
    return outs


def _prep_weights(inp):
    w1f, b1f = _fold_bn(inp['conv1_w'], inp['conv1_b'], inp['bn1_g'],
                        inp['bn1_b'], inp['bn1_m'], inp['bn1_v'])
    w2f, b2f = _fold_bn(inp['conv2_w'], inp['conv2_b'], inp['bn2_g'],
                        inp['bn2_b'], inp['bn2_m'], inp['bn2_v'])
    w3f, b3f = _fold_bn(inp['conv3_w'], inp['conv3_b'], inp['bn3_g'],
                        inp['bn3_b'], inp['bn3_m'], inp['bn3_v'])
    w1h, w1l = _split(w1f)
    w2h, w2l = _split(w2f)
    w3h, w3l = _split(w3f)

    d = {}
    d['w1ph'] = _arr1(w1h.astype(np.float32)).astype(bf16)
    d['w1pl'] = _arr1(w1l.astype(np.float32)).astype(bf16)
    d['b1v'] = np.tile(b1f, 4).reshape(128, 1)
    d['w2ph'] = np.stack([_arr2(w2h.astype(np.float32), i)
                          for i in range(3)]).astype(bf16)
    d['w2pl'] = np.stack([_arr2(w2l.astype(np.float32), i)
                          for i in range(3)]).astype(bf16)
    d['b2v'] = np.tile(b2f, 2).reshape(128, 1)
    p1, p2 = [], []
    for dy in range(3):
        for dx in range(3):
            p1.append(_arr3(w3h.astype(np.float32), dy, dx))
            p2.append(np.concatenate([_arr3(w3l.astype(np.float32), dy, dx),
                                      _arr3(w3h.astype(np.float32), dy, dx)], 0))
    d['w3p1'] = np.stack(p1).astype(bf16)          # [9, 64, 128]
    d['w3p2'] = np.stack(p2).astype(bf16)          # [9, 128, 128]
    d['b3v'] = b3f.reshape(128, 1)
    # gate / experts (fold the 1/16 avgpool into gate_w and w1)
    d['gw'] = (inp['gate_w'] / 16.0).astype(np.float32)        # [128, 8]
    d['gb'] = inp['gate_b'].reshape(1, 8).astype(np.float32)
    d['w1e'] = np.ascontiguousarray(
        (inp['w1'] / 16.0).transpose(1, 0, 2)).astype(np.float32)  # [128,8,64]
    d['b1row'] = inp['b1'].reshape(1, 8, 64).astype(np.float32)
    d['w2e'] = np.ascontiguousarray(
        inp['w2'].transpose(1, 0, 2)).astype(np.float32)       # [64, 8, 10]
    d['b2e'] = inp['b2'].astype(np.float32)                    # [8, 10]
    return d


# ---------------------------------------------------------------- device IR

def _build_nc(debug=False):
    nc = bacc.Bacc("TRN2", target_bir_lowering=False, debug=False,
                   enable_asserts=True, num_devices=N_CORES)

    xqh_d = nc.dram_tensor("xqh", [108, BC, 256], BF16,
                           kind="ExternalInput").ap()
    xql_d = nc.dram_tensor("xql", [108, BC, 256], BF16,
                           kind="ExternalInput").ap()
    wd = {}
    for name, shape, dt in [
            ('w1ph', [108, 128], BF16), ('w1pl', [108, 128], BF16),
            ('b1v', [128, 1], F32),
            ('w2ph', [3, 128, 128], BF16), ('w2pl', [3, 128, 128], BF16),
            ('b2v', [128, 1], F32),
            ('w3p1', [9, 64, 128], BF16), ('w3p2', [9, 128, 128], BF16),
            ('b3v', [128, 1], F32),
            ('gw', [128, 8], F32), ('gb', [1, 8], F32),
            ('w1e', [128, 8, 64], F32), ('b1row', [1, 8, 64], F32),
            ('w2e', [64, 8, 10], F32), ('b2e', [8, 10], F32)]:
        wd[name] = nc.dram_tensor(name, shape, dt, kind="ExternalInput").ap()
    out_d = nc.dram_tensor("out", [BC, 10], F32, kind="ExternalOutput").ap()
    feat_d = None
    if debug:
        feat_d = nc.dram_tensor("featT", [128, BC], F32, kind="ExternalOutput").ap()

    Relu = mybir.ActivationFunctionType.Relu
    Exp = mybir.ActivationFunctionType.Exp

    with tile.TileContext(nc) as tc:
        with tc.tile_pool(name="persist", bufs=1) as pp, \
             tc.tile_pool(name="work", bufs=3) as wp, \
             tc.tile_pool(name="ps", bufs=2, space="PSUM") as psp:

            # --- persistent SBUF tensors
            xq1h = pp.tile([108, MEGA, 256], BF16)
            xq1l = pp.tile([108, MEGA, 256], BF16)
            C1q = pp.tile([128, MEGA, 4, 16], F32)
            bands2f = pp.tile([128, 8, MEGA, 18], F32)
            bands2h = pp.tile([128, 8, MEGA, 18], BF16)
            bands2l = pp.tile([128, 8, MEGA, 18], BF16)
            xp3 = pp.tile([128, MEGA, 10, 10], BF16)
            lo3 = pp.tile([64, MEGA, 10, 10], BF16)
            featT = pp.tile([128, BC], F32)
            ident = pp.tile([128, 128], F32)
            ones_t = pp.tile([1, 128], F32)

            nc.vector.memset(bands2f[:], 0.0)
            nc.vector.memset(xp3[:], 0.0)
            nc.vector.memset(lo3[:], 0.0)
            make_identity(nc, ident[:])
            nc.vector.memset(ones_t[:], 1.0)

            # --- weights to SBUF
            ws = {}
            for name, src in wd.items():
                v = src
                if name in ('w2ph', 'w2pl'):
                    v = src.rearrange("d p m -> p d m")
                elif name in ('w3p1', 'w3p2'):
                    v = src.rearrange("t p m -> p t m")
                t = pp.tile(list(v.shape), src.dtype, name="ws_" + name)
                nc.sync.dma_start(out=t[:], in_=v)
                ws[name] = t

            for mega in range(NMEGA):
                g0 = mega * MEGA
                # ---- conv1 im2col: two contiguous DMAs from host tensors
                nc.sync.dma_start(out=xq1h[:], in_=xqh_d[:, g0:g0 + MEGA, :])
                nc.sync.dma_start(out=xq1l[:], in_=xql_d[:, g0:g0 + MEGA, :])

                # ---- conv1 matmuls (3 split passes) + evict + pool1
                for s in range(MEGA // 2):
                    sl = slice(s * 2, (s + 1) * 2)
                    ps1 = psp.tile([128, 2, 256], F32, tag="psA")
                    nc.tensor.matmul(ps1[:], ws['w1ph'][:], xq1h[:, sl, :],
                                     start=True, stop=False)
                    nc.tensor.matmul(ps1[:], ws['w1pl'][:], xq1h[:, sl, :],
                                     start=False, stop=False)
                    nc.tensor.matmul(ps1[:], ws['w1ph'][:], xq1l[:, sl, :],
                                     start=False, stop=True)
                    c1o = wp.tile([128, 2, 8, 32], F32, tag="c1o")
                    nc.scalar.activation(c1o[:], ps1[:], Relu,
                                         bias=ws['b1v'][:], scale=1.0)
                    rm = wp.tile([128, 2, 4, 32], F32, tag="rm1")
                    nc.vector.tensor_max(rm[:], c1o[:, :, 0::2, :],
                                         c1o[:, :, 1::2, :])
                    nc.vector.tensor_max(C1q[:, sl, :, :],
                                         rm[:, :, :, 0::2], rm[:, :, :, 1::2])

                # ---- conv2 band assembly (per-band 3D DMAs)
                for rr in range(4):
                    for b2 in range(8):
                        yp = 2 * b2 - 1 + rr
                        if not (0 <= yp < 16):
                            continue
                        q, ry = yp // 4, yp % 4
                        nc.sync.dma_start(
                            out=bands2f[rr * 32:(rr + 1) * 32, b2, :, 1:17],
                            in_=C1q[q * 32:(q + 1) * 32, :, ry, :])

                # ---- conv2 + pool2 (s outer for conv3 overlap)
                for s2 in range(MEGA // 32):
                    sl2 = slice(s2 * 32, (s2 + 1) * 32)
                    for b2 in range(8):
                        if s2 == 0:
                            nc.vector.tensor_copy(bands2h[:, b2, :, :],
                                                  bands2f[:, b2, :, :])
                            nc.vector.tensor_sub(bands2l[:, b2, :, :],
                                                 bands2f[:, b2, :, :],
                                                 bands2h[:, b2, :, :])
                        ps2 = psp.tile([128, 32, 16], F32, tag="psB")
                        for dxi in range(3):
                            xw = slice(dxi, dxi + 16)
                            nc.tensor.matmul(ps2[:], ws['w2ph'][:, dxi, :],
                                             bands2h[:, b2, sl2, xw],
                                             start=(dxi == 0), stop=False)
                            nc.tensor.matmul(ps2[:], ws['w2pl'][:, dxi, :],
                                             bands2h[:, b2, sl2, xw],
                                             start=False, stop=False)
                            nc.tensor.matmul(ps2[:], ws['w2ph'][:, dxi, :],
                                             bands2l[:, b2, sl2, xw],
                                             start=False, stop=(dxi == 2))
                        c2o = wp.tile([128, 32, 16], F32, tag="c2o")
                        nc.scalar.activation(c2o[:], ps2[:], Relu,
                                             bias=ws['b2v'][:], scale=1.0)
                        mv2 = wp.tile([64, 32, 16], F32, tag="mv2")
                        nc.sync.dma_start(out=mv2[:], in_=c2o[64:128, :, :])
                        rm2 = wp.tile([64, 32, 16], F32, tag="rm2")
                        nc.vector.tensor_max(rm2[:], c2o[0:64, :, :], mv2[:])
                        cm2 = wp.tile([64, 32, 8], F32, tag="cm2")
                        nc.vector.tensor_max(cm2[:], rm2[:, :, 0::2],
                                             rm2[:, :, 1::2])
                        nc.vector.tensor_copy(xp3[0:64, sl2, b2 + 1, 1:9], cm2[:])
                        nc.vector.tensor_sub(lo3[:, sl2, b2 + 1, 1:9],
                                             cm2[:], xp3[0:64, sl2, b2 + 1, 1:9])
                    # move lo half into xp3 partitions 64-127 for this s2
                    nc.sync.dma_start(out=xp3[64:128, sl2, :, :],
                                      in_=lo3[:, sl2, :, :])
                    # ---- conv3 for this s2 (4 slices of 8 imgs)
                    for s3 in range(4):
                        g3 = slice(s2 * 32 + s3 * 8, s2 * 32 + s3 * 8 + 8)
                        ps3 = psp.tile([128, 8, 8, 8], F32, tag="psC")
                        for t in range(9):
                            dy, dx = t // 3, t % 3
                            nc.tensor.matmul(
                                ps3[:], ws['w3p1'][:, t, :],
                                xp3[0:64, g3, dy:dy + 8, dx:dx + 8],
                                start=(t == 0), stop=False)
                            nc.tensor.matmul(
                                ps3[:], ws['w3p2'][:, t, :],
                                xp3[0:128, g3, dy:dy + 8, dx:dx + 8],
                                start=False, stop=(t == 8))
                        c3o = wp.tile([128, 8, 8, 8], F32, tag="c3o")
                        nc.scalar.activation(c3o[:], ps3[:], Relu,
                                             bias=ws['b3v'][:], scale=1.0)
                        rm3 = wp.tile([128, 8, 4, 8], F32, tag="rm3")
                        nc.vector.tensor_max(rm3[:], c3o[:, :, 0::2, :],
                                             c3o[:, :, 1::2, :])
                        cm3 = wp.tile([128, 8, 4, 4], F32, tag="cm3")
                        nc.vector.tensor_max(cm3[:], rm3[:, :, :, 0::2],
                                             rm3[:, :, :, 1::2])
                        fsl = slice(g0 + s2 * 32 + s3 * 8,
                                    g0 + s2 * 32 + s3 * 8 + 8)
                        nc.vector.tensor_reduce(
                            featT[:, fsl], cm3[:],
                            axis=mybir.AxisListType.XY, op=mybir.AluOpType.add)

            if debug:
                nc.sync.dma_start(out=feat_d, in_=featT[:])

            # ---------------- MoE head (exact fp32)
            for blk in range(BC // 128):
                tsl = slice(blk * 128, (blk + 1) * 128)
                lgp = psp.tile([128, 8], F32, tag="psA")
                nc.tensor.matmul(lgp[:], featT[:, tsl], ws['gw'][:],
                                 start=True, stop=False)
                nc.tensor.matmul(lgp[:], ones_t[0:1, :], ws['gb'][:],
                                 start=False, stop=True)
                lg = wp.tile([128, 8], F32, tag="lg")
                nc.scalar.copy(lg[:], lgp[:])
                m1 = wp.tile([128, 1], F32, tag="m1")
                nc.vector.reduce_max(m1[:], lg[:], axis=mybir.AxisListType.X)
                sel1 = wp.tile([128, 8], F32, tag="sel1")
                nc.vector.tensor_scalar(sel1[:], lg[:], m1[:], None,
                                        op0=mybir.AluOpType.is_ge)
                tmp = wp.tile([128, 8], F32, tag="tmp8")
                nc.vector.scalar_tensor_tensor(
                    tmp[:], in0=sel1[:], scalar=-1e30, in1=lg[:],
                    op0=mybir.AluOpType.mult, op1=mybir.AluOpType.add)
                m2 = wp.tile([128, 1], F32, tag="m2")
                nc.vector.reduce_max(m2[:], tmp[:], axis=mybir.AxisListType.X)
                sel = wp.tile([128, 8], F32, tag="sel")
                nc.vector.tensor_scalar(sel[:], lg[:], m2[:], None,
                                        op0=mybir.AluOpType.is_ge)
                negm1 = wp.tile([128, 1], F32, tag="negm1")
                nc.vector.tensor_scalar_mul(negm1[:], m1[:], -1.0)
                ex = wp.tile([128, 8], F32, tag="ex")
                nc.scalar.activation(ex[:], lg[:], Exp, bias=negm1[:], scale=1.0)
                e2 = wp.tile([128, 8], F32, tag="e2")
                nc.vector.tensor_mul(e2[:], ex[:], sel[:])
                ssum = wp.tile([128, 1], F32, tag="ssum")
                nc.vector.reduce_sum(ssum[:], e2[:], axis=mybir.AxisListType.X)
                rcp = wp.tile([128, 1], F32, tag="rcp")
                nc.vector.reciprocal(rcp[:], ssum[:])
                wt = wp.tile([128, 8], F32, tag="wt")
                nc.vector.tensor_scalar(wt[:], e2[:], rcp[:], None,
                                        op0=mybir.AluOpType.mult)
                # wt.T via PE transpose
                wtp = psp.tile([8, 128], F32, tag="psB")
                nc.tensor.transpose(wtp[:], wt[:], ident[0:128, 0:128])
                wtT = wp.tile([8, 128], F32, tag="wtT")
                nc.scalar.copy(wtT[:], wtp[:])

                out_ps = psp.tile([128, 10], F32, tag="psC")
                for e in range(8):
                    hep = psp.tile([128, 64], F32, tag="psA")
                    nc.tensor.matmul(hep[:], featT[:, tsl], ws['w1e'][:, e, :],
                                     start=True, stop=False)
                    nc.tensor.matmul(hep[:], ones_t[0:1, :],
                                     ws['b1row'][0:1, e, :],
                                     start=False, stop=True)
                    he = wp.tile([128, 64], F32, tag="he")
                    nc.scalar.activation(he[:], hep[:], Relu, scale=1.0)
                    hes = wp.tile([128, 64], F32, tag="hes")
                    nc.vector.tensor_scalar(hes[:], he[:], wt[:, e:e + 1], None,
                                            op0=mybir.AluOpType.mult)
                    hTp = psp.tile([64, 128], F32, tag="psB")
                    nc.tensor.transpose(hTp[:], hes[:], ident[:])
                    hT = wp.tile([64, 128], F32, tag="hT")
                    nc.scalar.copy(hT[:], hTp[:])
                    nc.tensor.matmul(out_ps[:], hT[:], ws['w2e'][:, e, :],
                                     start=(e == 0), stop=False)
                nc.tensor.matmul(out_ps[:], wtT[:], ws['b2e'][:],
                                 start=False, stop=True)
                outS = wp.tile([128, 10], F32, tag="outS")
                nc.scalar.copy(outS[:], out_ps[:])
                nc.sync.dma_start(out=out_d[tsl, :], in_=outS[:])

    nc.compile()
    return nc


# ---------------------------------------------------------------- entry

def kernel(**inputs):
    global last_result
    debug = bool(int(os.environ.get("KERNEL_DEBUG", "0")))
    key = ("nc", debug)
    if key not in _cache:
        _cache[key] = _build_nc(debug=debug)
    nc = _cache[key]

    w = _prep_weights(inputs)
    x = np.asarray(inputs['x'], np.float32)
    xqh, xql = _build_xq(x)  # [108, B, 256] bf16 each

    in_maps = []
    for c in range(N_CORES):
        sl = slice(c * BC, (c + 1) * BC)
        m = {'xqh': np.ascontiguousarray(xqh[:, sl]),
             'xql': np.ascontiguousarray(xql[:, sl])}
        for k, v in w.items():
            m[k] = v
        in_maps.append(m)

    trace = bool(int(os.environ.get("KERNEL_TRACE", "0")))
    res = run_bass_kernel_spmd(nc, in_maps, core_ids=list(range(N_CORES)),
                               trace=trace)
    last_result = res
    out = np.concatenate([res.results[c]["out"] for c in range(N_CORES)], 0)
    return out.astype(np.float32)


# revision 17
# speedup vs baseline: 1.3366x; 1.3366x over previous
"""Trainium2 Bass kernel for CNN backbone + top-2 MoE head (B=4096).

Data-parallel over 8 NeuronCores (512 images each). Convs are computed as
PE matmuls with split-bf16 (hi/lo) operands for fp32-grade accuracy:
  conv1: host-built quarter im2col (K=108: 4 row-quarters x 27 taps),
         M=128 (4 quarters x 32 out-ch); 3 split passes.
  conv2: row bands (K=128: 4 pooled rows x 32 ch), M=128 (2 out-rows x
         64 out-ch, yloc-major); 3 dx passes x 3 split terms; row-pool
         via DMA partition move + aligned max.
  conv3: 9-tap accumulation (K=64), M=128; 2 passes per tap via hi/lo
         stacking in partitions.
BN is folded into conv weights/biases host-side. Maxpools run on DVE via
strided tensor_max; gate + experts run in exact fp32 on the PE.
"""
import os
import numpy as np
import ml_dtypes

import concourse.bass as bass
import concourse.mybir as mybir
import concourse.tile as tile
from concourse import bacc
from concourse.bass_utils import run_bass_kernel_spmd
from concourse.masks import make_identity

F32 = mybir.dt.float32
BF16 = mybir.dt.bfloat16

N_CORES = 8
B_FULL = 4096
BC = B_FULL // N_CORES      # 512 images per core
MEGA = 32                   # images per pipeline chunk
NMEGA = BC // MEGA
BN_EPS = 1e-5

bf16 = ml_dtypes.bfloat16

_cache = {}
last_result = None


# ---------------------------------------------------------------- host prep

def _fold_bn(w, b, g, beta, mean, var):
    inv = g / np.sqrt(var + BN_EPS)
    wf = w * inv[:, None, None, None]
    bf_ = (b - mean) * inv + beta
    return wf.astype(np.float32), bf_.astype(np.float32)


def _split(a):
    hi = a.astype(bf16)
    lo = (a - hi.astype(np.float32)).astype(bf16)
    return hi, lo


def _arr1(w):
    """conv1 lhsT [108, 128]: p=(q*27 + c*9 + dy*3 + dx), m=(q*32 + o)."""
    out = np.zeros((108, 128), np.float32)
    for q in range(4):
        for c in range(3):
            for dy in range(3):
                for dx in range(3):
                    out[q * 27 + c * 9 + dy * 3 + dx, q * 32:(q + 1) * 32] = \
                        w[:, c, dy, dx]
    return out


def _arr2(w, dxi):
    """conv2 lhsT [128, 128]: p=(rr*32 + c), m=(yloc*64 + o)."""
    out = np.zeros((128, 128), np.float32)
    for rr in range(4):
        for c in range(32):
            for yloc in range(2):
                dy = rr - yloc
                if 0 <= dy <= 2:
                    out[rr * 32 + c, yloc * 64:(yloc + 1) * 64] = w[:, c, dy, dxi]
    return out


def _arr3(w, dy, dx):
    """conv3 per-tap lhsT [64, 128]: p=c, m=o."""
    return np.ascontiguousarray(w[:, :, dy, dx].T)  # [c, o]


def _build_xq(x):
    """Host-side conv1 quarter im2col: two [108, B, 256] bf16 (hi, lo).

    Partition p = q*27 + c*9 + dy*3 + dx; column n = ry*32 + xx within
    quarter q (out pixel y = q*8+ry); value = x[b, c, y+dy-1, xx+dx-1],
    zero-padded.
    """
    n = x.shape[0]
    xh = x.astype(bf16).astype(np.float32)
    xl = (x - xh).astype(bf16).astype(np.float32)
    outs = []
    for xv in (xh, xl):
        xpad = np.zeros((n, 3, 34, 34), np.float32)
        xpad[:, :, 1:33, 1:33] = xv
        xq = np.zeros((108, n, 256), np.float32)
        for q in range(4):
            for c in range(3):
                for dy in range(3):
                    for dx in range(3):
                        p = q * 27 + c * 9 + dy * 3 + dx
                        sl = xpad[:, c, q * 8 + dy:q * 8 + dy + 8, dx:dx + 32]
                        xq[p] = sl.reshape(n, 256)
        outs.append(xq.astype(bf16))
    return outs


def _prep_weights(inp):
    w1f, b1f = _fold_bn(inp['conv1_w'], inp['conv1_b'], inp['bn1_g'],
                        inp['bn1_b'], inp['bn1_m'], inp['bn1_v'])
    w2f, b2f = _fold_bn(inp['conv2_w'], inp['conv2_b'], inp['bn2_g'],
                        inp['bn2_b'], inp['bn2_m'], inp['bn2_v'])
    w3f, b3f = _fold_bn(inp['conv3_w'], inp['conv3_b'], inp['bn3_g'],
                        inp['bn3_b'], inp['bn3_m'], inp['bn3_v'])
    w1h, w1l = _split(w1f)
    w2h, w2l = _split(w2f)
    w3h, w3l = _split(w3f)

    d = {}
    d['w1ph'] = _arr1(w1h.astype(np.float32)).astype(bf16)
    d['w1pl'] = _arr1(w1l.astype(np.float32)).astype(bf16)
    d['b1v'] = np.tile(b1f, 4).reshape(128, 1)
    d['w2ph'] = np.stack([_arr2(w2h.astype(np.float32), i)
                          for i in range(3)]).astype(bf16)
    d['w2pl'] = np.stack([_arr2(w2l.astype(np.float32), i)
                          for i in range(3)]).astype(bf16)
    d['b2v'] = np.tile(b2f, 2).reshape(128, 1)
    p1, p2 = [], []
    for dy in range(3):
        for dx in range(3):
            p1.append(_arr3(w3h.astype(np.float32), dy, dx))
            p2.append(np.concatenate([_arr3(w3l.astype(np.float32), dy, dx),
                                      _arr3(w3h.astype(np.float32), dy, dx)], 0))
    d['w3p1'] = np.stack(p1).astype(bf16)          # [9, 64, 128]
    d['w3p2'] = np.stack(p2).astype(bf16)          # [9, 128, 128]
    d['b3v'] = b3f.reshape(128, 1)
    # gate / experts (fold the 1/16 avgpool into gate_w and w1)
    d['gw'] = (inp['gate_w'] / 16.0).astype(np.float32)        # [128, 8]
    d['gb'] = inp['gate_b'].reshape(1, 8).astype(np.float32)
    d['w1e'] = np.ascontiguousarray(
        (inp['w1'] / 16.0).transpose(1, 0, 2)).astype(np.float32)  # [128,8,64]
    d['b1row'] = inp['b1'].reshape(1, 8, 64).astype(np.float32)
    d['w2e'] = np.ascontiguousarray(
        inp['w2'].transpose(1, 0, 2)).astype(np.float32)       # [64, 8, 10]
    d['b2e'] = inp['b2'].astype(np.float32)                    # [8, 10]
    return d


# ---------------------------------------------------------------- device IR

def _build_nc(debug=False):
    nc = bacc.Bacc("TRN2", target_bir_lowering=False, debug=False,
                   enable_asserts=True, num_devices=N_CORES)

    xqh_d = nc.dram_tensor("xqh", [108, BC, 256], BF16,
                           kind="ExternalInput").ap()
    xql_d = nc.dram_tensor("xql", [108, BC, 256], BF16,
                           kind="ExternalInput").ap()
    wd = {}
    for name, shape, dt in [
            ('w1ph', [108, 128], BF16), ('w1pl', [108, 128], BF16),
            ('b1v', [128, 1], F32),
            ('w2ph', [3, 128, 128], BF16), ('w2pl', [3, 128, 128], BF16),
            ('b2v', [128, 1], F32),
            ('w3p1', [9, 64, 128], BF16), ('w3p2', [9, 128, 128], BF16),
            ('b3v', [128, 1], F32),
            ('gw', [128, 8], F32), ('gb', [1, 8], F32),
            ('w1e', [128, 8, 64], F32), ('b1row', [1, 8, 64], F32),
            ('w2e', [64, 8, 10], F32), ('b2e', [8, 10], F32)]:
        wd[name] = nc.dram_tensor(name, shape, dt, kind="ExternalInput").ap()
    out_d = nc.dram_tensor("out", [BC, 10], F32, kind="ExternalOutput").ap()
    feat_d = None
    if debug:
        feat_d = nc.dram_tensor("featT", [128, BC], F32, kind="ExternalOutput").ap()

    Relu = mybir.ActivationFunctionType.Relu
    Exp = mybir.ActivationFunctionType.Exp

    with tile.TileContext(nc) as tc:
        with tc.tile_pool(name="persist", bufs=1) as pp, \
             tc.tile_pool(name="work", bufs=3) as wp, \
             tc.tile_pool(name="ps", bufs=2, space="PSUM") as psp:

            # --- persistent SBUF tensors
            xq1h = pp.tile([108, MEGA, 256], BF16)
            xq1l = pp.tile([108, MEGA, 256], BF16)
            C1q = pp.tile([128, MEGA, 4, 16], F32)
            bands2f = pp.tile([128, 8, MEGA, 18], F32)
            bands2h = pp.tile([128, 8, MEGA, 18], BF16)
            bands2l = pp.tile([128, 8, MEGA, 18], BF16)
            xp3 = pp.tile([128, MEGA, 10, 10], BF16)
            lo3 = pp.tile([64, MEGA, 10, 10], BF16)
            featT = pp.tile([128, BC], F32)
            c2all = pp.tile([128, 8, MEGA, 16], F32)
            mv2 = pp.tile([64, 8, MEGA, 16], F32)
            rm2 = pp.tile([64, 8, MEGA, 16], F32)
            cm2f = pp.tile([64, 8, MEGA, 8], F32)
            ident = pp.tile([128, 128], F32)
            ones_t = pp.tile([1, 128], F32)

            nc.vector.memset(bands2f[:], 0.0)
            nc.vector.memset(xp3[:], 0.0)
            nc.vector.memset(lo3[:], 0.0)
            make_identity(nc, ident[:])
            nc.vector.memset(ones_t[:], 1.0)

            # --- weights to SBUF
            ws = {}
            for name, src in wd.items():
                v = src
                if name in ('w2ph', 'w2pl'):
                    v = src.rearrange("d p m -> p d m")
                elif name in ('w3p1', 'w3p2'):
                    v = src.rearrange("t p m -> p t m")
                t = pp.tile(list(v.shape), src.dtype, name="ws_" + name)
                nc.sync.dma_start(out=t[:], in_=v)
                ws[name] = t

            def emit_conv3(m):
                for s3 in range(MEGA // 8):
                    g3 = slice(s3 * 8, (s3 + 1) * 8)
                    ps3 = psp.tile([128, 8, 8, 8], F32, tag="psC")
                    for t in range(9):
                        dy, dx = t // 3, t % 3
                        nc.tensor.matmul(
                            ps3[:], ws['w3p1'][:, t, :],
                            xp3[0:64, g3, dy:dy + 8, dx:dx + 8],
                            start=(t == 0), stop=False)
                    for t in range(9):
                        dy, dx = t // 3, t % 3
                        nc.tensor.matmul(
                            ps3[:], ws['w3p2'][:, t, :],
                            xp3[0:128, g3, dy:dy + 8, dx:dx + 8],
                            start=False, stop=(t == 8))
                    c3o = wp.tile([128, 8, 8, 8], F32, tag="c3o")
                    nc.scalar.activation(c3o[:], ps3[:], Relu,
                                         bias=ws['b3v'][:], scale=1.0)
                    rm3 = wp.tile([128, 8, 4, 8], F32, tag="rm3")
                    nc.vector.tensor_max(rm3[:], c3o[:, :, 0::2, :],
                                         c3o[:, :, 1::2, :])
                    cm3 = wp.tile([128, 8, 4, 4], F32, tag="cm3")
                    nc.vector.tensor_max(cm3[:], rm3[:, :, :, 0::2],
                                         rm3[:, :, :, 1::2])
                    fsl = slice(m * MEGA + s3 * 8, m * MEGA + s3 * 8 + 8)
                    nc.vector.tensor_reduce(
                        featT[:, fsl], cm3[:],
                        axis=mybir.AxisListType.XY, op=mybir.AluOpType.add)

            for mega in range(NMEGA):
                g0 = mega * MEGA
                # ---- conv1 im2col: two contiguous DMAs from host tensors
                nc.sync.dma_start(out=xq1h[:], in_=xqh_d[:, g0:g0 + MEGA, :])
                nc.sync.dma_start(out=xq1l[:], in_=xql_d[:, g0:g0 + MEGA, :])

                # ---- conv1 matmuls (3 split passes) + evict + pool1
                for s in range(MEGA // 2):
                    sl = slice(s * 2, (s + 1) * 2)
                    ps1 = psp.tile([128, 2, 256], F32, tag="psA")
                    nc.tensor.matmul(ps1[:], ws['w1ph'][:], xq1h[:, sl, :],
                                     start=True, stop=False)
                    nc.tensor.matmul(ps1[:], ws['w1pl'][:], xq1h[:, sl, :],
                                     start=False, stop=False)
                    nc.tensor.matmul(ps1[:], ws['w1ph'][:], xq1l[:, sl, :],
                                     start=False, stop=True)
                    c1o = wp.tile([128, 2, 8, 32], F32, tag="c1o")
                    nc.scalar.activation(c1o[:], ps1[:], Relu,
                                         bias=ws['b1v'][:], scale=1.0)
                    rm = wp.tile([128, 2, 4, 32], F32, tag="rm1")
                    nc.vector.tensor_max(rm[:], c1o[:, :, 0::2, :],
                                         c1o[:, :, 1::2, :])
                    nc.vector.tensor_max(C1q[:, sl, :, :],
                                         rm[:, :, :, 0::2], rm[:, :, :, 1::2])

                # ---- conv2 band assembly (per-band 3D DMAs)
                for rr in range(4):
                    for b2 in range(8):
                        yp = 2 * b2 - 1 + rr
                        if not (0 <= yp < 16):
                            continue
                        q, ry = yp // 4, yp % 4
                        nc.sync.dma_start(
                            out=bands2f[rr * 32:(rr + 1) * 32, b2, :, 1:17],
                            in_=C1q[q * 32:(q + 1) * 32, :, ry, :])

                # ---- conv3 of the previous mega (pipelined)
                if mega > 0:
                    emit_conv3(mega - 1)

                # ---- conv2 (whole mega; batched pool2 afterwards)
                for b2 in range(8):
                    nc.vector.tensor_copy(bands2h[:, b2, :, :],
                                          bands2f[:, b2, :, :])
                    nc.gpsimd.tensor_sub(bands2l[:, b2, :, :],
                                         bands2f[:, b2, :, :],
                                         bands2h[:, b2, :, :])
                    ps2 = psp.tile([128, MEGA, 16], F32, tag="psB")
                    for dxi in range(3):
                        xw = slice(dxi, dxi + 16)
                        nc.tensor.matmul(ps2[:], ws['w2ph'][:, dxi, :],
                                         bands2h[:, b2, :, xw],
                                         start=(dxi == 0), stop=False)
                        nc.tensor.matmul(ps2[:], ws['w2pl'][:, dxi, :],
                                         bands2h[:, b2, :, xw],
                                         start=False, stop=False)
                        nc.tensor.matmul(ps2[:], ws['w2ph'][:, dxi, :],
                                         bands2l[:, b2, :, xw],
                                         start=False, stop=(dxi == 2))
                    nc.scalar.activation(c2all[:, b2, :, :], ps2[:], Relu,
                                         bias=ws['b2v'][:], scale=1.0)
                # pool2: one partition-move DMA + whole-mega max/colmax/split
                nc.sync.dma_start(out=mv2[:], in_=c2all[64:128, :, :, :])
                nc.vector.tensor_max(rm2[:], c2all[0:64, :, :, :], mv2[:])
                xp3v = xp3[0:64, :, 1:9, 1:9].rearrange("p g r x -> p r g x")
                lo3v = lo3[:, :, 1:9, 1:9].rearrange("p g r x -> p r g x")
                nc.vector.tensor_max(xp3v, rm2[:, :, :, 0::2],
                                     rm2[:, :, :, 1::2])
                nc.vector.tensor_max(cm2f[:], rm2[:, :, :, 0::2],
                                     rm2[:, :, :, 1::2])
                nc.gpsimd.tensor_sub(lo3v, cm2f[:],
                                     xp3[0:64, :, 1:9, 1:9].rearrange(
                                         "p g r x -> p r g x"))
                nc.sync.dma_start(out=xp3[64:128, :, :, :], in_=lo3[:])

            # ---- trailing conv3 for the last mega
            emit_conv3(NMEGA - 1)

            if debug:
                nc.sync.dma_start(out=feat_d, in_=featT[:])

            # ---------------- MoE head (exact fp32)
            for blk in range(BC // 128):
                tsl = slice(blk * 128, (blk + 1) * 128)
                lgp = psp.tile([128, 8], F32, tag="psA")
                nc.tensor.matmul(lgp[:], featT[:, tsl], ws['gw'][:],
                                 start=True, stop=False)
                nc.tensor.matmul(lgp[:], ones_t[0:1, :], ws['gb'][:],
                                 start=False, stop=True)
                lg = wp.tile([128, 8], F32, tag="lg")
                nc.scalar.copy(lg[:], lgp[:])
                m1 = wp.tile([128, 1], F32, tag="m1")
                nc.vector.reduce_max(m1[:], lg[:], axis=mybir.AxisListType.X)
                sel1 = wp.tile([128, 8], F32, tag="sel1")
                nc.vector.tensor_scalar(sel1[:], lg[:], m1[:], None,
                                        op0=mybir.AluOpType.is_ge)
                tmp = wp.tile([128, 8], F32, tag="tmp8")
                nc.vector.scalar_tensor_tensor(
                    tmp[:], in0=sel1[:], scalar=-1e30, in1=lg[:],
                    op0=mybir.AluOpType.mult, op1=mybir.AluOpType.add)
                m2 = wp.tile([128, 1], F32, tag="m2")
                nc.vector.reduce_max(m2[:], tmp[:], axis=mybir.AxisListType.X)
                sel = wp.tile([128, 8], F32, tag="sel")
                nc.vector.tensor_scalar(sel[:], lg[:], m2[:], None,
                                        op0=mybir.AluOpType.is_ge)
                negm1 = wp.tile([128, 1], F32, tag="negm1")
                nc.vector.tensor_scalar_mul(negm1[:], m1[:], -1.0)
                ex = wp.tile([128, 8], F32, tag="ex")
                nc.scalar.activation(ex[:], lg[:], Exp, bias=negm1[:], scale=1.0)
                e2 = wp.tile([128, 8], F32, tag="e2")
                nc.vector.tensor_mul(e2[:], ex[:], sel[:])
                ssum = wp.tile([128, 1], F32, tag="ssum")
                nc.vector.reduce_sum(ssum[:], e2[:], axis=mybir.AxisListType.X)
                rcp = wp.tile([128, 1], F32, tag="rcp")
                nc.vector.reciprocal(rcp[:], ssum[:])
                wt = wp.tile([128, 8], F32, tag="wt")
                nc.vector.tensor_scalar(wt[:], e2[:], rcp[:], None,
                                        op0=mybir.AluOpType.mult)
                # wt.T via PE transpose
                wtp = psp.tile([8, 128], F32, tag="psB")
                nc.tensor.transpose(wtp[:], wt[:], ident[0:128, 0:128])
                wtT = wp.tile([8, 128], F32, tag="wtT")
                nc.scalar.copy(wtT[:], wtp[:])

                out_ps = psp.tile([128, 10], F32, tag="psC")
                for e in range(8):
                    hep = psp.tile([128, 64], F32, tag="psA")
                    nc.tensor.matmul(hep[:], featT[:, tsl], ws['w1e'][:, e, :],
                                     start=True, stop=False)
                    nc.tensor.matmul(hep[:], ones_t[0:1, :],
                                     ws['b1row'][0:1, e, :],
                                     start=False, stop=True)
                    he = wp.tile([128, 64], F32, tag="he")
                    nc.scalar.activation(he[:], hep[:], Relu, scale=1.0)
                    hes = wp.tile([128, 64], F32, tag="hes")
                    nc.vector.tensor_scalar(hes[:], he[:], wt[:, e:e + 1], None,
                                            op0=mybir.AluOpType.mult)
                    hTp = psp.tile([64, 128], F32, tag="psB")
                    nc.tensor.transpose(hTp[:], hes[:], ident[:])
                    hT = wp.tile([64, 128], F32, tag="hT")
                    nc.scalar.copy(hT[:], hTp[:])
                    nc.tensor.matmul(out_ps[:], hT[:], ws['w2e'][:, e, :],
                                     start=(e == 0), stop=False)
                nc.tensor.matmul(out_ps[:], wtT[:], ws['b2e'][:],
                                 start=False, stop=True)
                outS = wp.tile([128, 10], F32, tag="outS")
                nc.scalar.copy(outS[:], out_ps[:])
                nc.sync.dma_start(out=out_d[tsl, :], in_=outS[:])

    nc.compile()
    return nc


# ---------------------------------------------------------------- entry

def kernel(**inputs):
    global last_result
    debug = bool(int(os.environ.get("KERNEL_DEBUG", "0")))
    key = ("nc", debug)
    if key not in _cache:
        _cache[key] = _build_nc(debug=debug)
    nc = _cache[key]

    w = _prep_weights(inputs)
    x = np.asarray(inputs['x'], np.float32)
    xqh, xql = _build_xq(x)  # [108, B, 256] bf16 each

    in_maps = []
    for c in range(N_CORES):
        sl = slice(c * BC, (c + 1) * BC)
        m = {'xqh': np.ascontiguousarray(xqh[:, sl]),
             'xql': np.ascontiguousarray(xql[:, sl])}
        for k, v in w.items():
            m[k] = v
        in_maps.append(m)

    trace = bool(int(os.environ.get("KERNEL_TRACE", "0")))
    res = run_bass_kernel_spmd(nc, in_maps, core_ids=list(range(N_CORES)),
                               trace=trace)
    last_result = res
    out = np.concatenate([res.results[c]["out"] for c in range(N_CORES)], 0)
    return out.astype(np.float32)
